# revision 51
# baseline (speedup 1.0000x reference)
"""Adaptive per-pixel Gaussian smoothing (7x7, sigma from a sigmoid of a
perspective map) on 8 Trainium2 NeuronCores.

Strategy (v3, pixel-major, ~3.4x over the v1 channel-major kernel)
------------------------------------------------------------------
Shard: data-parallel over (batch, H-half): 4 batches x 2 halves = 8 cores.

Layout: partitions = the core's 128 image ROWS; free dim = (channel, col),
cols padded +-3. The per-pixel weight maps u_d [128 rows, 256 cols] are
host-precomputed (sigma/exp chain in numpy), packed into one [128, 10, W]
bf16 tensor, and broadcast across the 64 channels with 0-stride APs -- no
PE broadcast matmuls and no on-device transcendental preamble.

Math: w[i,j](p) = e1(p)^(i^2+j^2) * invS2(p), e1 = exp(-1/(2 sigma^2)),
invS2 = (sum_i e1^(i^2))^-2. With vertical tap sums U_a = x(y-a)+x(y+a)
(PE banded-selector matmuls over the row/partition axis; halo rows from a
6-row tile) and horizontal shifts as free-dim AP offsets:
    T_{a,b} = U_a << b + U_a >> b,  C_d = sum of T_{a,b} with a^2+b^2 = d,
    out = sum_d u_d * C_d   (10 distinct d in {0,1,2,4,5,8,9,10,13,18}).
The d-sum is accumulated by PE identity matmuls into PSUM (fp32), ACT
stages to SBUF, DMA out.

Engine split (per 8-channel sub-block, all tensor ops bf16 so DVE runs in
its 2x_1p fast mode): DVE does the grouped T/R adds and most muls in-place;
Pool (GPSIMD) the three diagonal muls + M13; the otherwise-idle DMA engines
compute R1, R2 and the b=3 T-group as copy + accumulate-add transfer pairs
(SWDGE); PE does vertical taps (pipelined one sub-block ahead) + the d-sum.
"""

import numpy as np
import ml_dtypes

import concourse.bass as bass
import concourse.tile as tile
from concourse import mybir
from concourse.bass_utils import run_bass_kernel_spmd

F32 = mybir.dt.float32
BF16 = mybir.dt.bfloat16
AF = mybir.ActivationFunctionType
OP = mybir.AluOpType

B, C, H, W = 4, 64, 256, 256
NCORES = 8
HS = H // 2          # 128 rows per core
WP = W + 6           # 262 padded cols
PAD = 3
CB = 8               # channels per sub-block
NSB = C // CB        # 8 sub-blocks
LN2 = 0.6931471805599453

DS = [0, 1, 4, 9, 2, 5, 8, 10, 13, 18]   # emission order
# ring composition: d -> list of building blocks, see _build_nc
_CACHE = {}


def _build_nc():
    nc = bass.Bass()
    xm_in = nc.declare_dram_parameter("xm", [HS, C, WP], BF16, isOutput=False)
    xh_in = nc.declare_dram_parameter("xh", [6, C, WP], BF16, isOutput=False)
    um_in = nc.declare_dram_parameter("umaps", [128, 10, W], BF16, isOutput=False)
    selv_in = nc.declare_dram_parameter("selv", [128, 3, 128], BF16, isOutput=False)
    selh_in = nc.declare_dram_parameter("selh", [6, 3, 128], BF16, isOutput=False)
    id_in = nc.declare_dram_parameter("ident", [128, 128], BF16, isOutput=False)
    out_d = nc.declare_dram_parameter("out", [HS, C, W], BF16, isOutput=True)

    with tile.TileContext(nc) as tc:
        with (
            tc.tile_pool(name="const", bufs=1) as constp,
            tc.tile_pool(name="maps", bufs=1) as mapsp,
            tc.tile_pool(name="xp", bufs=1) as xp_,
            tc.tile_pool(name="ua", bufs=2) as uap,
            tc.tile_pool(name="tr", bufs=1) as trp,
            tc.tile_pool(name="ob", bufs=2) as obp,
            tc.tile_pool(name="psu", bufs=1, space="PSUM") as psu,
            tc.tile_pool(name="psa", bufs=1, space="PSUM") as psa,
        ):
            # ---------- DMAs: umaps first (gates Pool's first muls),
            # then PE's U-production inputs (selv, selh, xh, xm chunk0)
            umaps = mapsp.tile([128, 10, W], BF16, tag="umaps", name="umaps")
            nc.scalar.dma_start(umaps[:], um_in[:])
            selv = constp.tile([128, 3, 128], BF16, tag="selv", name="selv")
            nc.scalar.dma_start(selv[:], selv_in[:])
            selh = constp.tile([6, 3, 128], BF16, tag="selh", name="selh")
            nc.scalar.dma_start(selh[:], selh_in[:])
            ident = constp.tile([128, 128], BF16, tag="ident", name="ident")
            nc.scalar.dma_start(ident[:], id_in[:])
            xh = xp_.tile([6, C, WP], BF16, tag="xh", name="xh")
            nc.scalar.dma_start(xh[:], xh_in[:])
            xm = xp_.tile([HS, C, WP], BF16, tag="xm", name="xm")

            def xm_load(g):
                nc.scalar.dma_start(xm[:, 8 * g:8 * (g + 1), :],
                                    xm_in[:, 8 * g:8 * (g + 1), :])

            xm_load(0)
            xm_load(1)

            u149 = umaps[:, 0:3, :]
            udig = umaps[:, 3:6, :]
            u510 = umaps[:, 6:8, :]
            u13v = umaps[:, 8:9, :]
            u0v = umaps[:, 9:10, :]

            def ub1(ap):    # [128, 1, W] -> [128, CB, W]
                return ap.squeeze(1).unsqueeze(1).broadcast_to([128, CB, W])

            def ubg(ap, k):  # [128, k, W] -> [128, k, CB, W]
                return ap.unsqueeze(2).broadcast_to([128, k, CB, W])

            # persistent double-buffered U_a tiles with once-zeroed col pads:
            # ua_all[p][:, a-1, c, x] = x(y-a, c, x) + x(y+a, c, x)
            ua_all = []
            for p_ in range(2):
                t = mapsp.tile([128, 3, CB, WP], BF16, tag=f"uaall{p_}",
                               name=f"uaall{p_}")
                nc.gpsimd.memset(t[:, :, :, 0:PAD], 0.0)
                nc.gpsimd.memset(t[:, :, :, PAD + W:WP], 0.0)
                ua_all.append(t)

            # persistent grouped work tiles (in-place consumers).
            # t9a[(b-1)*3 + (a-1)] = T_{a,b} = U_a << b + U_a >> b, b in 1,2
            # t3x[p][a-1] = T_{a,3}; rx[p][b-1] = R_b (DMA-written: 2 bufs)
            t9as = [trp.tile([128, 6, CB, W], BF16, tag=f"t9a{p_}",
                             name=f"t9a{p_}") for p_ in range(2)]
            t3x = [trp.tile([128, 3, CB, W], BF16, tag=f"t3x{p_}",
                            name=f"t3x{p_}") for p_ in range(2)]
            rxs = [trp.tile([128, 3, CB, W], BF16, tag=f"rx{p_}",
                            name=f"rx{p_}") for p_ in range(2)]

            # ---------- main loop over 8-channel sub-blocks ----------

            def emit_U(cb):
                """PE vertical taps U_a of sub-block cb into PSUM, ACT copy
                to the padded persistent SBUF tiles (parity cb%2)."""
                c0 = cb * CB
                ups = psu.tile([128, CB, W], F32, tag="ups", name="ups")
                for a in (1, 2, 3):
                    for j in range(CB // 2):
                        cj = c0 + 2 * j
                        nc.tensor.matmul(
                            ups[:, 2 * j:2 * j + 2, :],
                            selv[:, a - 1, :],
                            xm[:, cj:cj + 2, PAD:PAD + W],
                            start=True, stop=False, skip_group_check=True)
                        nc.tensor.matmul(
                            ups[:, 2 * j:2 * j + 2, :],
                            selh[:, a - 1, :],
                            xh[:, cj:cj + 2, PAD:PAD + W],
                            start=False, stop=True, skip_group_check=True)
                    nc.scalar.copy(
                        ua_all[cb % 2][:, a - 1, :, PAD:PAD + W], ups[:])

            def cxc(cb):
                return xm[:, cb * CB:(cb + 1) * CB, :]

            def cxsh(cb, b):
                x_ = cxc(cb)
                return (x_[:, :, PAD - b:PAD - b + W].unsqueeze(1),
                        x_[:, :, PAD + b:PAD + b + W].unsqueeze(1))

            def cush(cb, b):
                u_ = ua_all[cb % 2]
                return (u_[:, :, :, PAD - b:PAD - b + W],
                        u_[:, :, :, PAD + b:PAD + b + W])

            def emit_copies(cb):
                """first halves of the DMA-engine adds (HWDGE, scalar)"""
                rx_, t3_ = rxs[cb % 2], t3x[cb % 2]
                nc.scalar.dma_start(t3_[:], cush(cb, 3)[0])
                if cb > 0:
                    nc.scalar.dma_start(rx_[:, 0:1], cxsh(cb, 1)[0])
                    nc.scalar.dma_start(rx_[:, 1:2], cxsh(cb, 2)[0])

            def emit_accums(cb):
                """second halves: SWDGE accumulate-adds (gpsimd-issued;
                emitted after Pool's muls so the issue's wait on the copy
                never head-of-line blocks them)"""
                rx_, t3_ = rxs[cb % 2], t3x[cb % 2]
                nc.gpsimd.dma_start(t3_[:], cush(cb, 3)[1], accum_op=OP.add)
                if cb > 0:
                    nc.gpsimd.dma_start(rx_[:, 0:1], cxsh(cb, 1)[1],
                                        accum_op=OP.add)
                    nc.gpsimd.dma_start(rx_[:, 1:2], cxsh(cb, 2)[1],
                                        accum_op=OP.add)

            emit_U(0)
            emit_copies(0)
            emit_accums(0)
            V, P = nc.vector, nc.gpsimd
            for cb in range(NSB):
                c0 = cb * CB
                # prefetch the x chunk two sub-blocks ahead, then produce
                # next sub-block's U while DVE/Pool grind this one
                # (PE is in-order: these must precede cb's d-sum matmuls)
                if cb + 2 < NSB:
                    xm_load(cb + 2)
                if cb + 1 < NSB:
                    emit_U(cb + 1)
                    emit_copies(cb + 1)
                ua = ua_all[cb % 2]
                t3 = t3x[cb % 2]
                rx = rxs[cb % 2]
                t9a = t9as[cb % 2]

                xc = xm[:, c0:c0 + CB, :]
                accA = psa.tile([128, 4, W], F32, tag="accA", name="accA")
                accB = psa.tile([128, 4, W], F32, tag="accB", name="accB")
                nacc = [0]

                def acc(tm_ap, first=False, last=False):
                    """accumulate one d-term [128, CB, W] into the two
                    PSUM halves via identity matmuls"""
                    for q0, acc_ in ((0, accA), (4, accB)):
                        for q in range(2):
                            nc.tensor.matmul(
                                acc_[:, 2 * q:2 * q + 2, :],
                                ident[:],
                                tm_ap[:, q0 + 2 * q:q0 + 2 * q + 2, :],
                                start=first, stop=last,
                                skip_group_check=True)
                    nacc[0] += 1

                def sh(b, lo=0, hi=3):
                    """col-shifted [128, hi-lo, CB, W] views of padded ua"""
                    return (ua[:, lo:hi, :, PAD - b:PAD - b + W],
                            ua[:, lo:hi, :, PAD + b:PAD + b + W])

                def xsh(b):
                    return (xc[:, :, PAD - b:PAD - b + W].unsqueeze(1),
                            xc[:, :, PAD + b:PAD + b + W].unsqueeze(1))

                # --- DVE stream (bf16 2x mode, ~0.53ns/el)
                if cb == 0:
                    s0, s1 = xsh(1)
                    V.tensor_add(rx[:, 0:1], s0, s1)       # R1 (DVE idle)
                    s0, s1 = xsh(2)
                    V.tensor_add(rx[:, 1:2], s0, s1)       # R2
                if cb == 0:
                    # per-a singles: start as each ua_a copy lands instead
                    # of waiting for all three
                    for a_ in (1, 2, 3):
                        s0, s1 = sh(1, a_ - 1, a_)
                        V.tensor_add(t9a[:, a_ - 1:a_], s0, s1)
                        s0, s1 = sh(2, a_ - 1, a_)
                        V.tensor_add(t9a[:, a_ + 2:a_ + 3], s0, s1)
                else:
                    s0, s1 = sh(1)
                    V.tensor_add(t9a[:, 0:3], s0, s1)      # T11,T21,T31
                    s0, s1 = sh(2)
                    V.tensor_add(t9a[:, 3:6], s0, s1)      # T12,T22,T32
                s0, s1 = xsh(3)
                V.tensor_add(rx[:, 2:3], s0, s1)           # R3
                # C149 = R_b + U_b (in place), then M149 = C149 * u_{1,4,9}
                V.tensor_add(rx[:], rx[:], ua[:, :, :, PAD:PAD + W])
                V.tensor_mul(rx[:], rx[:], ubg(u149, 3))
                # C5 = T12+T21 -> t9a[3]; then M5
                V.tensor_add(t9a[:, 3:4], t9a[:, 3:4], t9a[:, 1:2])
                V.tensor_mul(t9a[:, 3:4], t9a[:, 3:4],
                             ubg(u510[:, 0:1, :], 1))
                # M0 = xc * u0 in place (after the R's consumed xc)
                V.tensor_mul(xc[:, :, PAD:PAD + W], xc[:, :, PAD:PAD + W],
                             ub1(u0v))
                # C13 = T23+T32 -> t3[1] (late: give the t3 DMA-pair slack)
                V.tensor_add(t3[:, 1:2], t3[:, 1:2], t9a[:, 5:6])
                # C10 = T13+T31 -> t3[0]; then M10
                V.tensor_add(t3[:, 0:1], t3[:, 0:1], t9a[:, 2:3])
                V.tensor_mul(t3[:, 0:1], t3[:, 0:1],
                             ubg(u510[:, 1:2, :], 1))

                # --- Pool: diagonal muls + M13 (terminal, feed only PE).
                # On the last sub-block DVE drains first, so give it the
                # two tail muls to cut the kernel's serial tail.
                E2 = V if cb == NSB - 1 else P

                def pmul(eng, dst, k_):
                    eng.tensor_mul(dst, dst,
                                   ubg(udig[:, k_:k_ + 1, :], 1))
                pmul(P, t9a[:, 0:1], 0)                    # d=2
                pmul(P, t9a[:, 4:5], 1)                    # d=8
                pmul(E2, t3[:, 2:3], 2)                    # d=18
                E2.tensor_mul(t3[:, 1:2], t3[:, 1:2],
                              ubg(u13v, 1))  # d=13
                if cb + 1 < NSB:
                    emit_accums(cb + 1)

                # --- PE accumulation, in approximate completion order
                def sq(ap):
                    return ap.squeeze(1)

                acc(sq(t9a[:, 0:1]), first=True)           # d=2
                acc(sq(t9a[:, 4:5]))                       # d=8
                for k in range(3):                         # d=1,4,9
                    acc(sq(rx[:, k:k + 1]))
                acc(xc[:, :, PAD:PAD + W])                 # d=0
                acc(sq(t9a[:, 3:4]))                       # d=5
                acc(sq(t3[:, 2:3]))                        # d=18
                acc(sq(t3[:, 1:2]))                        # d=13
                acc(sq(t3[:, 0:1]), last=True)             # d=10
                assert nacc[0] == 10

                # --- stage out of PSUM and store
                for hi, acc_ in enumerate((accA, accB)):
                    osb = obp.tile([128, 4, W], BF16, tag=f"osb{hi}",
                                   name=f"osb{hi}", bufs=1)
                    nc.scalar.copy(osb[:], acc_[:])
                    nc.scalar.dma_start(
                        out_d[:, c0 + 4 * hi:c0 + 4 * hi + 4, :], osb[:])
    return nc


def _selectors():
    if "selv" not in _CACHE:
        selv = np.zeros((128, 3, 128), ml_dtypes.bfloat16)
        selh = np.zeros((6, 3, 128), ml_dtypes.bfloat16)
        for ai, a in enumerate((1, 2, 3)):
            for m in range(128):
                if m - a >= 0:
                    selv[m - a, ai, m] = 1
                if m + a < 128:
                    selv[m + a, ai, m] = 1
            # halo rows: k 0..2 = image rows -3..-1; k 3..5 = rows 128..130
            for k in range(3):
                r = k - 3
                if 0 <= r + a < 128:
                    selh[k, ai, r + a] = 1
            for k in range(3, 6):
                r = 128 + k - 3
                if 0 <= r - a < 128:
                    selh[k, ai, r - a] = 1
        _CACHE["selv"] = selv
        _CACHE["selh"] = selh
    return _CACHE["selv"], _CACHE["selh"]


def _get_nc():
    if "nc" not in _CACHE:
        nc = _build_nc()
        _split_waits(nc)
        _CACHE["nc"] = nc
    return _CACHE["nc"]


def _split_waits(nc):
    """Walrus on this toolchain accepts only one semaphore wait per compute
    instruction; hoist excess waits onto same-engine NoOps placed before."""
    for f in nc.m.functions:
        for bb in f.blocks:
            new_list = []
            for ins in bb.instructions:
                si = ins.sync_info
                if si is not None and len(si.on_wait) > 1:
                    waits = list(si.on_wait)
                    for k, w in enumerate(waits[:-1]):
                        nop = mybir.InstNoOp(name=f"{ins.name}-ws{k}",
                                             ins=[], outs=[])
                        nop.engine = ins.engine
                        nop.sync_info = mybir.SyncInfo(on_wait=[w], on_update=[])
                        new_list.append(nop)
                    ins.sync_info = mybir.SyncInfo(on_wait=[waits[-1]],
                                                  on_update=list(si.on_update))
                new_list.append(ins)
            bb.instructions = new_list


def _umaps(persp_core, a, bt, gm):
    """host-side per-pixel weight maps, packed
    [u1,u4,u9 | u2,u8,u18 | u5,u10 | u13 | u0] -> [128, 10, W] bf16"""
    sg = 1.0 / (1.0 + np.exp(-(bt * persp_core + gm), dtype=np.float32))
    sigma = np.maximum(a * sg, np.float32(1e-4))
    e1 = np.exp(-1.0 / (2.0 * sigma * sigma), dtype=np.float32)
    s = 1.0 + 2.0 * (e1 + e1 ** 4 + e1 ** 9)
    u0 = (1.0 / (s * s)).astype(np.float32)
    um = np.empty((HS, 10, W), np.float32)
    for k, d in enumerate((1, 4, 9, 2, 8, 18, 5, 10, 13)):
        um[:, k, :] = (e1 ** d) * u0
    um[:, 9, :] = u0
    return um.astype(ml_dtypes.bfloat16)


def _in_maps(x, perspective, a, bt, gm):
    selv, selh = _selectors()
    ident = np.eye(128, dtype=ml_dtypes.bfloat16)

    xp = np.pad(x, ((0, 0), (0, 0), (PAD, PAD), (PAD, PAD)))
    in_maps = []
    for b in range(B):
        for half in range(2):
            r0 = half * HS
            # rows r0-3 .. r0+130 of the padded image = xp rows r0 .. r0+133
            sl = xp[b, :, r0:r0 + HS + 6, :]           # [C, 134, WP] f32
            slt = np.ascontiguousarray(
                sl.transpose(1, 0, 2)).astype(ml_dtypes.bfloat16)
            xm = np.ascontiguousarray(slt[3:3 + HS])   # [128, C, WP]
            xhalo = np.ascontiguousarray(
                np.concatenate([slt[0:3], slt[3 + HS:6 + HS]], 0))
            in_maps.append({
                "xm": xm,
                "xh": xhalo,
                "umaps": _umaps(perspective[b, 0, r0:r0 + HS, :], a, bt, gm),
                "selv": selv,
                "selh": selh,
                "ident": ident,
            })
    return in_maps


def kernel(x, perspective, alpha, beta, gamma, kernel_size):
    assert int(kernel_size) == 7
    x = np.asarray(x, dtype=np.float32)
    perspective = np.asarray(perspective, dtype=np.float32)
    a = np.float32(np.asarray(alpha).reshape(-1)[0])
    bt = np.float32(np.asarray(beta).reshape(-1)[0])
    gm = np.float32(np.asarray(gamma).reshape(-1)[0])

    in_maps = _in_maps(x, perspective, a, bt, gm)
    nc = _get_nc()
    res = run_bass_kernel_spmd(nc, in_maps, list(range(NCORES)))
    _CACHE["last_res"] = res
    out = np.empty((B, C, H, W), np.float32)
    k = 0
    for b in range(B):
        for half in range(2):
            out[b, :, half * HS:(half + 1) * HS, :] = \
                res.results[k]["out"].transpose(1, 0, 2)
            k += 1
    return out


if __name__ == "__main__":
    rng = np.random.default_rng(0)
    x = rng.standard_normal((B, C, H, W)).astype(np.float32)
    persp = rng.random((B, 1, H, W)).astype(np.float32)
    o = kernel(x=x, perspective=persp, alpha=np.ones(1, np.float32) * 3,
               beta=np.ones(1, np.float32), gamma=np.zeros(1, np.float32),
               kernel_size=7)
    print(o.shape, o.dtype, float(np.abs(o).mean()))


# revision 57
# speedup vs baseline: 1.0014x; 1.0014x over previous
"""Adaptive per-pixel Gaussian smoothing (7x7, sigma from a sigmoid of a
perspective map) on 8 Trainium2 NeuronCores.

Strategy (v3, pixel-major, ~3.4x over the v1 channel-major kernel)
------------------------------------------------------------------
Shard: data-parallel over (batch, H-half): 4 batches x 2 halves = 8 cores.

Layout: partitions = the core's 128 image ROWS; free dim = (channel, col),
cols padded +-3. The per-pixel weight maps u_d [128 rows, 256 cols] are
host-precomputed (sigma/exp chain in numpy), packed into one [128, 10, W]
bf16 tensor, and broadcast across the 64 channels with 0-stride APs -- no
PE broadcast matmuls and no on-device transcendental preamble.

Math: w[i,j](p) = e1(p)^(i^2+j^2) * invS2(p), e1 = exp(-1/(2 sigma^2)),
invS2 = (sum_i e1^(i^2))^-2. With vertical tap sums U_a = x(y-a)+x(y+a)
(PE banded-selector matmuls over the row/partition axis; halo rows from a
6-row tile) and horizontal shifts as free-dim AP offsets:
    T_{a,b} = U_a << b + U_a >> b,  C_d = sum of T_{a,b} with a^2+b^2 = d,
    out = sum_d u_d * C_d   (10 distinct d in {0,1,2,4,5,8,9,10,13,18}).
The d-sum is accumulated by PE identity matmuls into PSUM (fp32), ACT
stages to SBUF, DMA out.

Engine split (per 8-channel sub-block, all tensor ops bf16 so DVE runs in
its 2x_1p fast mode): DVE does the grouped T/R adds and most muls in-place;
Pool (GPSIMD) the three diagonal muls + M13; the otherwise-idle DMA engines
compute R1, R2 and the b=3 T-group as copy + accumulate-add transfer pairs
(SWDGE); PE does vertical taps (pipelined one sub-block ahead) + the d-sum.
"""

import numpy as np
import ml_dtypes

import concourse.bass as bass
import concourse.tile as tile
from concourse import mybir
from concourse.bass_utils import run_bass_kernel_spmd

F32 = mybir.dt.float32
BF16 = mybir.dt.bfloat16
AF = mybir.ActivationFunctionType
OP = mybir.AluOpType

B, C, H, W = 4, 64, 256, 256
NCORES = 8
HS = H // 2          # 128 rows per core
WP = W + 6           # 262 padded cols
PAD = 3
CB = 8               # channels per sub-block
NSB = C // CB        # 8 sub-blocks
LN2 = 0.6931471805599453

DS = [0, 1, 4, 9, 2, 5, 8, 10, 13, 18]   # emission order
# ring composition: d -> list of building blocks, see _build_nc
_CACHE = {}


def _build_nc():
    nc = bass.Bass()
    xm_in = nc.declare_dram_parameter("xm", [HS, C, WP], BF16, isOutput=False)
    xh_in = nc.declare_dram_parameter("xh", [6, C, WP], BF16, isOutput=False)
    um_in = nc.declare_dram_parameter("umaps", [128, 10, W], BF16, isOutput=False)
    cst_in = nc.declare_dram_parameter("csts", [128, 7, 128], BF16, isOutput=False)
    out_d = nc.declare_dram_parameter("out", [HS, C, W], BF16, isOutput=True)

    with tile.TileContext(nc) as tc:
        with (
            tc.tile_pool(name="const", bufs=1) as constp,
            tc.tile_pool(name="maps", bufs=1) as mapsp,
            tc.tile_pool(name="xp", bufs=1) as xp_,
            tc.tile_pool(name="ua", bufs=2) as uap,
            tc.tile_pool(name="tr", bufs=1) as trp,
            tc.tile_pool(name="ob", bufs=2) as obp,
            tc.tile_pool(name="psu", bufs=1, space="PSUM") as psu,
            tc.tile_pool(name="psa", bufs=1, space="PSUM") as psa,
        ):
            # ---------- DMAs: umaps first (gates Pool's first muls),
            # then PE's U-production inputs (selv, selh, xh, xm chunk0)
            umaps = mapsp.tile([128, 10, W], BF16, tag="umaps", name="umaps")
            nc.scalar.dma_start(umaps[:], um_in[:])
            # selectors + identity packed as one DMA (one issue latency on
            # the serial track instead of three, ahead of the x chunks)
            csts = constp.tile([128, 7, 128], BF16, tag="csts", name="csts")
            nc.scalar.dma_start(csts[:], cst_in[:])
            selv = csts[:, 0:3, :]
            selh = csts[0:6, 3:6, :]
            ident = csts[:, 6, :]
            xh = xp_.tile([6, C, WP], BF16, tag="xh", name="xh")
            nc.scalar.dma_start(xh[:], xh_in[:])
            xm = xp_.tile([HS, C, WP], BF16, tag="xm", name="xm")

            def xm_load(g):
                nc.scalar.dma_start(xm[:, 8 * g:8 * (g + 1), :],
                                    xm_in[:, 8 * g:8 * (g + 1), :])

            xm_load(0)
            xm_load(1)

            u149 = umaps[:, 0:3, :]
            udig = umaps[:, 3:6, :]
            u510 = umaps[:, 6:8, :]
            u13v = umaps[:, 8:9, :]
            u0v = umaps[:, 9:10, :]

            def ub1(ap):    # [128, 1, W] -> [128, CB, W]
                return ap.squeeze(1).unsqueeze(1).broadcast_to([128, CB, W])

            def ubg(ap, k):  # [128, k, W] -> [128, k, CB, W]
                return ap.unsqueeze(2).broadcast_to([128, k, CB, W])

            # persistent double-buffered U_a tiles with once-zeroed col pads:
            # ua_all[p][:, a-1, c, x] = x(y-a, c, x) + x(y+a, c, x)
            ua_all = []
            for p_ in range(2):
                t = mapsp.tile([128, 3, CB, WP], BF16, tag=f"uaall{p_}",
                               name=f"uaall{p_}")
                nc.gpsimd.memset(t[:, :, :, 0:PAD], 0.0)
                nc.gpsimd.memset(t[:, :, :, PAD + W:WP], 0.0)
                ua_all.append(t)

            # persistent grouped work tiles (in-place consumers).
            # t9a[(b-1)*3 + (a-1)] = T_{a,b} = U_a << b + U_a >> b, b in 1,2
            # t3x[p][a-1] = T_{a,3}; rx[p][b-1] = R_b (DMA-written: 2 bufs)
            t9as = [trp.tile([128, 6, CB, W], BF16, tag=f"t9a{p_}",
                             name=f"t9a{p_}") for p_ in range(2)]
            t3x = [trp.tile([128, 3, CB, W], BF16, tag=f"t3x{p_}",
                            name=f"t3x{p_}") for p_ in range(2)]
            rxs = [trp.tile([128, 3, CB, W], BF16, tag=f"rx{p_}",
                            name=f"rx{p_}") for p_ in range(2)]

            # ---------- main loop over 8-channel sub-blocks ----------

            def emit_U(cb):
                """PE vertical taps U_a of sub-block cb into PSUM, ACT copy
                to the padded persistent SBUF tiles (parity cb%2)."""
                c0 = cb * CB
                ups = psu.tile([128, CB, W], F32, tag="ups", name="ups")
                for a in (1, 2, 3):
                    for j in range(CB // 2):
                        cj = c0 + 2 * j
                        nc.tensor.matmul(
                            ups[:, 2 * j:2 * j + 2, :],
                            selv[:, a - 1, :],
                            xm[:, cj:cj + 2, PAD:PAD + W],
                            start=True, stop=False, skip_group_check=True)
                        nc.tensor.matmul(
                            ups[:, 2 * j:2 * j + 2, :],
                            selh[:, a - 1, :],
                            xh[:, cj:cj + 2, PAD:PAD + W],
                            start=False, stop=True, skip_group_check=True)
                    nc.scalar.copy(
                        ua_all[cb % 2][:, a - 1, :, PAD:PAD + W], ups[:])

            def cxc(cb):
                return xm[:, cb * CB:(cb + 1) * CB, :]

            def cxsh(cb, b):
                x_ = cxc(cb)
                return (x_[:, :, PAD - b:PAD - b + W].unsqueeze(1),
                        x_[:, :, PAD + b:PAD + b + W].unsqueeze(1))

            def cush(cb, b):
                u_ = ua_all[cb % 2]
                return (u_[:, :, :, PAD - b:PAD - b + W],
                        u_[:, :, :, PAD + b:PAD + b + W])

            def emit_copies(cb):
                """first halves of the DMA-engine adds (HWDGE, scalar)"""
                rx_, t3_ = rxs[cb % 2], t3x[cb % 2]
                nc.scalar.dma_start(t3_[:], cush(cb, 3)[0])
                if cb > 0:
                    nc.scalar.dma_start(rx_[:, 0:1], cxsh(cb, 1)[0])
                    nc.scalar.dma_start(rx_[:, 1:2], cxsh(cb, 2)[0])

            def emit_accums(cb):
                """second halves: SWDGE accumulate-adds (gpsimd-issued;
                emitted after Pool's muls so the issue's wait on the copy
                never head-of-line blocks them)"""
                rx_, t3_ = rxs[cb % 2], t3x[cb % 2]
                nc.gpsimd.dma_start(t3_[:], cush(cb, 3)[1], accum_op=OP.add)
                if cb > 0:
                    nc.gpsimd.dma_start(rx_[:, 0:1], cxsh(cb, 1)[1],
                                        accum_op=OP.add)
                    nc.gpsimd.dma_start(rx_[:, 1:2], cxsh(cb, 2)[1],
                                        accum_op=OP.add)

            emit_U(0)
            emit_copies(0)
            emit_accums(0)
            V, P = nc.vector, nc.gpsimd
            for cb in range(NSB):
                c0 = cb * CB
                # prefetch the x chunk two sub-blocks ahead, then produce
                # next sub-block's U while DVE/Pool grind this one
                # (PE is in-order: these must precede cb's d-sum matmuls)
                if cb + 2 < NSB:
                    xm_load(cb + 2)
                if cb + 1 < NSB:
                    emit_U(cb + 1)
                    emit_copies(cb + 1)
                ua = ua_all[cb % 2]
                t3 = t3x[cb % 2]
                rx = rxs[cb % 2]
                t9a = t9as[cb % 2]

                xc = xm[:, c0:c0 + CB, :]
                accA = psa.tile([128, 4, W], F32, tag="accA", name="accA")
                accB = psa.tile([128, 4, W], F32, tag="accB", name="accB")
                nacc = [0]

                def acc(tm_ap, first=False, last=False):
                    """accumulate one d-term [128, CB, W] into the two
                    PSUM halves via identity matmuls"""
                    for q0, acc_ in ((0, accA), (4, accB)):
                        for q in range(2):
                            nc.tensor.matmul(
                                acc_[:, 2 * q:2 * q + 2, :],
                                ident,
                                tm_ap[:, q0 + 2 * q:q0 + 2 * q + 2, :],
                                start=first, stop=last,
                                skip_group_check=True)
                    nacc[0] += 1

                def sh(b, lo=0, hi=3):
                    """col-shifted [128, hi-lo, CB, W] views of padded ua"""
                    return (ua[:, lo:hi, :, PAD - b:PAD - b + W],
                            ua[:, lo:hi, :, PAD + b:PAD + b + W])

                def xsh(b):
                    return (xc[:, :, PAD - b:PAD - b + W].unsqueeze(1),
                            xc[:, :, PAD + b:PAD + b + W].unsqueeze(1))

                # --- DVE stream (bf16 2x mode, ~0.53ns/el)
                if cb == 0:
                    s0, s1 = xsh(1)
                    V.tensor_add(rx[:, 0:1], s0, s1)       # R1 (DVE idle)
                    s0, s1 = xsh(2)
                    V.tensor_add(rx[:, 1:2], s0, s1)       # R2
                if cb == 0:
                    # per-a singles: start as each ua_a copy lands instead
                    # of waiting for all three
                    for a_ in (1, 2, 3):
                        s0, s1 = sh(1, a_ - 1, a_)
                        V.tensor_add(t9a[:, a_ - 1:a_], s0, s1)
                        s0, s1 = sh(2, a_ - 1, a_)
                        V.tensor_add(t9a[:, a_ + 2:a_ + 3], s0, s1)
                else:
                    s0, s1 = sh(1)
                    V.tensor_add(t9a[:, 0:3], s0, s1)      # T11,T21,T31
                    s0, s1 = sh(2)
                    V.tensor_add(t9a[:, 3:6], s0, s1)      # T12,T22,T32
                s0, s1 = xsh(3)
                V.tensor_add(rx[:, 2:3], s0, s1)           # R3
                # C149 = R_b + U_b (in place), then M149 = C149 * u_{1,4,9}
                V.tensor_add(rx[:], rx[:], ua[:, :, :, PAD:PAD + W])
                V.tensor_mul(rx[:], rx[:], ubg(u149, 3))
                # C5 = T12+T21 -> t9a[3]; then M5
                V.tensor_add(t9a[:, 3:4], t9a[:, 3:4], t9a[:, 1:2])
                V.tensor_mul(t9a[:, 3:4], t9a[:, 3:4],
                             ubg(u510[:, 0:1, :], 1))
                # M0 = xc * u0 in place (after the R's consumed xc)
                V.tensor_mul(xc[:, :, PAD:PAD + W], xc[:, :, PAD:PAD + W],
                             ub1(u0v))
                # C13 = T23+T32 -> t3[1] (late: give the t3 DMA-pair slack)
                V.tensor_add(t3[:, 1:2], t3[:, 1:2], t9a[:, 5:6])
                # C10 = T13+T31 -> t3[0]; then M10
                V.tensor_add(t3[:, 0:1], t3[:, 0:1], t9a[:, 2:3])
                V.tensor_mul(t3[:, 0:1], t3[:, 0:1],
                             ubg(u510[:, 1:2, :], 1))

                # --- Pool: diagonal muls + M13 (terminal, feed only PE).
                # On the last sub-block DVE drains first, so give it the
                # two tail muls to cut the kernel's serial tail.
                E2 = V if cb == NSB - 1 else P

                def pmul(eng, dst, k_):
                    eng.tensor_mul(dst, dst,
                                   ubg(udig[:, k_:k_ + 1, :], 1))
                pmul(P, t9a[:, 0:1], 0)                    # d=2
                pmul(P, t9a[:, 4:5], 1)                    # d=8
                pmul(E2, t3[:, 2:3], 2)                    # d=18
                E2.tensor_mul(t3[:, 1:2], t3[:, 1:2],
                              ubg(u13v, 1))  # d=13
                if cb + 1 < NSB:
                    emit_accums(cb + 1)

                # --- PE accumulation, in approximate completion order
                def sq(ap):
                    return ap.squeeze(1)

                acc(sq(t9a[:, 0:1]), first=True)           # d=2
                acc(sq(t9a[:, 4:5]))                       # d=8
                for k in range(3):                         # d=1,4,9
                    acc(sq(rx[:, k:k + 1]))
                acc(xc[:, :, PAD:PAD + W])                 # d=0
                acc(sq(t9a[:, 3:4]))                       # d=5
                acc(sq(t3[:, 2:3]))                        # d=18
                acc(sq(t3[:, 1:2]))                        # d=13
                acc(sq(t3[:, 0:1]), last=True)             # d=10
                assert nacc[0] == 10

                # --- stage out of PSUM and store
                for hi, acc_ in enumerate((accA, accB)):
                    osb = obp.tile([128, 4, W], BF16, tag=f"osb{hi}",
                                   name=f"osb{hi}", bufs=1)
                    nc.scalar.copy(osb[:], acc_[:])
                    nc.scalar.dma_start(
                        out_d[:, c0 + 4 * hi:c0 + 4 * hi + 4, :], osb[:])
    return nc


def _selectors():
    if "selv" not in _CACHE:
        selv = np.zeros((128, 3, 128), ml_dtypes.bfloat16)
        selh = np.zeros((6, 3, 128), ml_dtypes.bfloat16)
        for ai, a in enumerate((1, 2, 3)):
            for m in range(128):
                if m - a >= 0:
                    selv[m - a, ai, m] = 1
                if m + a < 128:
                    selv[m + a, ai, m] = 1
            # halo rows: k 0..2 = image rows -3..-1; k 3..5 = rows 128..130
            for k in range(3):
                r = k - 3
                if 0 <= r + a < 128:
                    selh[k, ai, r + a] = 1
            for k in range(3, 6):
                r = 128 + k - 3
                if 0 <= r - a < 128:
                    selh[k, ai, r - a] = 1
        _CACHE["selv"] = selv
        _CACHE["selh"] = selh
    return _CACHE["selv"], _CACHE["selh"]


def _get_nc():
    if "nc" not in _CACHE:
        nc = _build_nc()
        _split_waits(nc)
        _CACHE["nc"] = nc
    return _CACHE["nc"]


def _split_waits(nc):
    """Walrus on this toolchain accepts only one semaphore wait per compute
    instruction; hoist excess waits onto same-engine NoOps placed before."""
    for f in nc.m.functions:
        for bb in f.blocks:
            new_list = []
            for ins in bb.instructions:
                si = ins.sync_info
                if si is not None and len(si.on_wait) > 1:
                    waits = list(si.on_wait)
                    for k, w in enumerate(waits[:-1]):
                        nop = mybir.InstNoOp(name=f"{ins.name}-ws{k}",
                                             ins=[], outs=[])
                        nop.engine = ins.engine
                        nop.sync_info = mybir.SyncInfo(on_wait=[w], on_update=[])
                        new_list.append(nop)
                    ins.sync_info = mybir.SyncInfo(on_wait=[waits[-1]],
                                                  on_update=list(si.on_update))
                new_list.append(ins)
            bb.instructions = new_list


def _umaps(persp_core, a, bt, gm):
    """host-side per-pixel weight maps, packed
    [u1,u4,u9 | u2,u8,u18 | u5,u10 | u13 | u0] -> [128, 10, W] bf16"""
    sg = 1.0 / (1.0 + np.exp(-(bt * persp_core + gm), dtype=np.float32))
    sigma = np.maximum(a * sg, np.float32(1e-4))
    e1 = np.exp(-1.0 / (2.0 * sigma * sigma), dtype=np.float32)
    s = 1.0 + 2.0 * (e1 + e1 ** 4 + e1 ** 9)
    u0 = (1.0 / (s * s)).astype(np.float32)
    um = np.empty((HS, 10, W), np.float32)
    for k, d in enumerate((1, 4, 9, 2, 8, 18, 5, 10, 13)):
        um[:, k, :] = (e1 ** d) * u0
    um[:, 9, :] = u0
    return um.astype(ml_dtypes.bfloat16)


def _in_maps(x, perspective, a, bt, gm):
    selv, selh = _selectors()
    csts = np.zeros((128, 7, 128), ml_dtypes.bfloat16)
    csts[:, 0:3, :] = selv
    csts[0:6, 3:6, :] = selh
    csts[:, 6, :] = np.eye(128, dtype=ml_dtypes.bfloat16)

    xp = np.pad(x, ((0, 0), (0, 0), (PAD, PAD), (PAD, PAD)))
    in_maps = []
    for b in range(B):
        for half in range(2):
            r0 = half * HS
            # rows r0-3 .. r0+130 of the padded image = xp rows r0 .. r0+133
            sl = xp[b, :, r0:r0 + HS + 6, :]           # [C, 134, WP] f32
            slt = np.ascontiguousarray(
                sl.transpose(1, 0, 2)).astype(ml_dtypes.bfloat16)
            xm = np.ascontiguousarray(slt[3:3 + HS])   # [128, C, WP]
            xhalo = np.ascontiguousarray(
                np.concatenate([slt[0:3], slt[3 + HS:6 + HS]], 0))
            in_maps.append({
                "xm": xm,
                "xh": xhalo,
                "umaps": _umaps(perspective[b, 0, r0:r0 + HS, :], a, bt, gm),
                "csts": csts,
            })
    return in_maps


def kernel(x, perspective, alpha, beta, gamma, kernel_size):
    assert int(kernel_size) == 7
    x = np.asarray(x, dtype=np.float32)
    perspective = np.asarray(perspective, dtype=np.float32)
    a = np.float32(np.asarray(alpha).reshape(-1)[0])
    bt = np.float32(np.asarray(beta).reshape(-1)[0])
    gm = np.float32(np.asarray(gamma).reshape(-1)[0])

    in_maps = _in_maps(x, perspective, a, bt, gm)
    nc = _get_nc()
    res = run_bass_kernel_spmd(nc, in_maps, list(range(NCORES)))
    _CACHE["last_res"] = res
    out = np.empty((B, C, H, W), np.float32)
    k = 0
    for b in range(B):
        for half in range(2):
            out[b, :, half * HS:(half + 1) * HS, :] = \
                res.results[k]["out"].transpose(1, 0, 2)
            k += 1
    return out


if __name__ == "__main__":
    rng = np.random.default_rng(0)
    x = rng.standard_normal((B, C, H, W)).astype(np.float32)
    persp = rng.random((B, 1, H, W)).astype(np.float32)
    o = kernel(x=x, perspective=persp, alpha=np.ones(1, np.float32) * 3,
               beta=np.ones(1, np.float32), gamma=np.zeros(1, np.float32),
               kernel_size=7)
    print(o.shape, o.dtype, float(np.abs(o).mean()))
